# revision 13
# baseline (speedup 1.0000x reference)
"""Trainium2 Bass kernel for nn_ARModel (AR(12) self-feeding recurrence).

Math: the reference scan is affine-linear in its initial history window
h0 = x[:, T-p:, :, 0] (the only part of x the output depends on):

    out[b, t, n] = sum_k W[t, n, k] * h0[b, n, k] + c[t, n]

where W (impulse-response coefficients) and c (bias response) depend only on
ar_params / bias and are unrolled on the host (weight preprocessing). This
removes the sequential T-scan from the device: the per-sample work becomes a
batch of tiny per-node matmuls.

Device mapping (per core, N sharded 8-ways -> 128 nodes/core):
  - groups of 2 nodes; per group one TensorE matmul
        out[64*i + b, t] = sum_{i,k} S[13*i + k, 64*i + b] * M[13*i + k, t]
    with S = block-diagonal h0 (plus a row of ones for the bias term) as the
    stationary operand and M = W rows (plus the c row) as the moving operand.
  - 4 groups live on partition strips {0,32,64,96} so DMA uses all 128
    partitions and matmuls run concurrently in separate PE row-groups
    (tile_position).
  - 64 matmuls of [26]x[128 x 288] -> PSUM [128, 288], copied to SBUF
    (DVE/ACT) and DMA'd out in 1.18MB chunks.
"""

import numpy as np

B, T, N, P = 64, 288, 1024, 12
NCORES = 8
NPC = N // NCORES  # nodes per core = 128
K = P + 1          # contraction rows per node (12 coeffs + 1 bias row)
JBLK = 16          # j index: 16 column blocks
STRIPS = 4         # partition strips at 0/32/64/96
GROUPS = JBLK * STRIPS          # 64 groups of 2 nodes per core
CHUNK_G = 8                     # groups per output DMA chunk
NCHUNK = GROUPS // CHUNK_G      # 8 chunks

_compiled = {}


def _build_bass():
    import concourse.mybir as mybir
    from concourse import bacc
    from concourse.tile import TileContext

    f32 = mybir.dt.float32
    bf16 = mybir.dt.bfloat16
    nc = bacc.Bacc("TRN2", target_bir_lowering=False)

    JW = 128 + T  # columns per j-block in the combined input: S (128) + M (288)
    i_d = nc.dram_tensor("inp", (128, JBLK * JW), bf16, kind="ExternalInput")
    o_d = nc.dram_tensor("out", (128, GROUPS * T), f32, kind="ExternalOutput")

    IN_CHUNK_J = 4  # j-blocks per input DMA

    with TileContext(nc) as tc:
        with (
            tc.tile_pool(name="consts", bufs=1) as cpool,
            tc.tile_pool(name="psum", bufs=8, space="PSUM") as ppool,
            tc.tile_pool(name="outp", bufs=3) as opool,
        ):
            in_sb = cpool.tile([128, JBLK * JW], bf16)
            for ic in range(JBLK // IN_CHUNK_J):
                cols = slice(ic * IN_CHUNK_J * JW, (ic + 1) * IN_CHUNK_J * JW)
                nc.sync.dma_start(in_sb[:, cols], i_d[:, cols])

            och = None
            for j in range(JBLK):
                if j % 2 == 0:
                    och = opool.tile([128, CHUNK_G * T], f32)
                for s in range(STRIPS):
                    goff = (j % 2) * STRIPS + s  # position inside output chunk
                    ps = ppool.tile([128, T], f32)
                    nc.tensor.matmul(
                        ps,
                        in_sb[32 * s : 32 * s + 2 * K, j * JW : j * JW + 128],
                        in_sb[32 * s : 32 * s + 2 * K, j * JW + 128 : (j + 1) * JW],
                        start=True,
                        stop=True,
                        tile_position=(32 * s, 0),
                    )
                    dst = och[:, goff * T : (goff + 1) * T]
                    if (j // 2) % 2 == 1:
                        nc.scalar.copy(dst, ps)
                    else:
                        nc.vector.tensor_copy(dst, ps)
                if j % 2 == 1:
                    nc.sync.dma_start(
                        o_d[:, (j - 1) * STRIPS * T : (j + 1) * STRIPS * T], och
                    )
    nc.finalize()
    return nc


def _unroll_weights(ar_params, bias):
    """Impulse-response unroll: W[t, n, k] = d s_t / d h0[k], c[t, n] = bias part."""
    a = ar_params.astype(np.float64)
    Wfull = np.zeros((T + P, N, P), np.float64)
    Wfull[np.arange(P), :, np.arange(P)] = 1.0
    c = np.zeros((T + P, N), np.float64)
    b64 = bias.astype(np.float64)
    for t in range(T):
        Wfull[P + t] = np.einsum("nj,jnk->nk", a, Wfull[t : t + P])
        c[P + t] = np.einsum("nj,jn->n", a, c[t : t + P]) + b64
    return Wfull[P:].astype(np.float32), c[P:].astype(np.float32)


def _pack_core(h0c, Wc, cc):
    """Build per-core DMA images.

    h0c: (B, P, 128)   last-P x slice for this core's nodes  [b, k, nl]
    Wc:  (T, 128, P)   [t, nl, k]
    cc:  (T, 128)      [t, nl]
    node index nl = 8*j + 2*s + i  (j in 0..15, s strip 0..3, i 0..1)
    """
    # moving operand: M[s, 13*i + k, j, t]
    Wr = Wc.transpose(1, 2, 0).reshape(JBLK, STRIPS, 2, P, T)  # (j, s, i, k, t)
    M = np.zeros((STRIPS, 2, K, JBLK, T), np.float32)
    M[:, :, :P] = Wr.transpose(1, 2, 3, 0, 4)
    ccr = cc.T.reshape(JBLK, STRIPS, 2, T)  # (j, s, i, t)
    M[:, :, P] = ccr.transpose(1, 2, 0, 3)
    m_pack = np.zeros((STRIPS, 32, JBLK, T), np.float32)
    m_pack[:, : 2 * K] = M.reshape(STRIPS, 2 * K, JBLK, T)

    # stationary operand: S[s, 13*i + k, j, 64*i + b] block-diagonal in i
    h0r = h0c.transpose(2, 1, 0).reshape(JBLK, STRIPS, 2, P, B)  # (j, s, i, k, b)
    S = np.zeros((STRIPS, 2, K, JBLK, 2, B), np.float32)
    hsk = h0r.transpose(1, 2, 3, 0, 4)  # (s, i, k, j, b)
    for i in range(2):
        S[:, i, :P, :, i, :] = hsk[:, i]
        S[:, i, P, :, i, :] = 1.0
    s_pack = np.zeros((STRIPS, 32, JBLK, 2 * B), np.float32)
    s_pack[:, : 2 * K] = S.reshape(STRIPS, 2 * K, JBLK, 2 * B)

    # combined per-j layout: [S_j (128 cols) | M_j (288 cols)]
    inp = np.concatenate([s_pack, m_pack], axis=3)  # (4, 32, 16, 416)
    import ml_dtypes

    return np.ascontiguousarray(inp).reshape(128, JBLK * (128 + T)).astype(
        ml_dtypes.bfloat16
    )


def kernel(x, ar_params, bias):
    from concourse import bass_utils

    x = np.ascontiguousarray(np.asarray(x, dtype=np.float32))
    ar_params = np.asarray(ar_params, dtype=np.float32)
    bias = np.asarray(bias, dtype=np.float32)

    W, c = _unroll_weights(ar_params, bias)
    h0 = x[:, T - P :, :, 0]  # (B, P, N)

    in_maps = []
    for ci in range(NCORES):
        sl = slice(ci * NPC, (ci + 1) * NPC)
        inp = _pack_core(h0[:, :, sl], W[:, sl, :], c[:, sl])
        in_maps.append({"inp": inp})

    if "nc" not in _compiled:
        _compiled["nc"] = _build_bass()
    res = bass_utils.run_bass_kernel_spmd(
        _compiled["nc"], in_maps, core_ids=list(range(NCORES))
    )
    _compiled["last_result"] = res  # exec_time_ns etc. when BASS_TRACE=1

    full = np.empty((B, T, N), np.float32)
    for ci in range(NCORES):
        r = res.results[ci]["out"].reshape(2, B, GROUPS, T)  # (i, b, g, t)
        blk = np.transpose(r, (1, 3, 2, 0))  # (b, t, g, i); nl = 2*g + i
        full[:, :, ci * NPC : (ci + 1) * NPC] = blk.reshape(B, T, NPC)
    return full[..., None]
